# revision 20
# baseline (speedup 1.0000x reference)
"""EfficientViT attention block on 8 TRN2 NeuronCores (v2).

Sharding: 8 cores = 4 images x 2 row-halves (64 rows each + halos).
s=1 cores get a vertically flipped image + dy-flipped conv weights so the
SPMD program is identical on all cores. kv partials are AllReduced pairwise.

v2 changes vs v1:
 - dw5 taps fused mul-add (scalar_tensor_tensor), split DVE/GPS/PE
   (PE taps fold the grouped-pw into block-diag weights, PSUM-accumulated
   with the pw matmul of the DVE/GPS accs).
 - ms kept in SBUF as one [128, 66, 384] tensor; Prelu epilogue masks q/k.
 - P4: relu-fused staging copies -> batched 4-row DMA transposes on two
   HWDGE rings -> strided-AP kv matmuls; head0 pipelined during dw5.
 - P6: one merged [36, cw] reciprocal per chunk; broadcasts via gpsimd.
 - P8/P9 hswish: min(x(x+3)/6, relu(x)) via ACT Square + one STT.
 - dw3: 4 PE taps (diag matmuls) + 5 DVE STT taps + identity fold.
"""
import numpy as np
import ml_dtypes

import concourse.bass as bass
import concourse.bacc as bacc
import concourse.tile as tile
from concourse import mybir
from concourse import bass_utils

F32 = mybir.dt.float32
BF16 = mybir.dt.bfloat16
Alu = mybir.AluOpType
AF = mybir.ActivationFunctionType
BF = ml_dtypes.bfloat16

BN_EPS = 1e-5
ATT_EPS = 1e-15
NCORES = 8

_CACHE = {}

ATT_SRCS = {0: (0, 1, 2), 1: (2, 3, 4), 2: (4, 5)}  # att9 tile a -> multi tiles

# dw5 tap split (tap = 5*dy + dx). DVE taps must be even-dx: odd element
# offsets break the DVE 2x packed mode (4B alignment), so odd-dx goes to PE.
DW5_PE = [(dy, dx) for dy in range(5) for dx in (1, 3)]          # 10 on PE
DW5_DVE = [(dy, dx) for dy in range(5) for dx in (0, 2, 4)]      # 15 on DVE

# dw3 tap split (tap = 3*dy + dx); same alignment rule
DW3_PE = [(0, 1), (1, 1), (2, 1), (1, 0)]                        # 4 on PE
DW3_DVE = [(0, 0), (0, 2), (1, 2), (2, 0), (2, 2)]               # 5 on DVE

HSW_A = 0.4082482904638631     # 1/sqrt(6)
HSW_B = 0.6123724356957945     # sqrt(6)/4
HSW_C = -0.375                 # -(HSW_B**2)


def q_chan(g, e):
    return 24 * g + e if g < 16 else 384 + 24 * (g - 16) + e


def _row_groups(nrows, gmax=16):
    out, r = [], 0
    while r < nrows:
        n = min(gmax, nrows - r)
        out.append((r, n))
        r += n
    return out


def _subs(nrows, gmax=4):
    return _row_groups(nrows, gmax)


def build_program():
    nc = bacc.Bacc("TRN2", target_bir_lowering=False, debug=False,
                   enable_asserts=False, num_devices=NCORES)
    d = {}
    def din(name, shape, dt):
        d[name] = nc.dram_tensor(name, shape, dt, kind="ExternalInput").ap()
    din("xr", [128, 72, 130], BF16)
    din("xo", [128, 72, 130], BF16)
    din("wc", [128, 27 * 128], BF16)
    din("cb", [128, 3], F32)
    din("cm", [128, 3], F32)
    din("dw5", [128, 75], F32)
    din("pww", [128, 3 * 128], BF16)
    din("wblk5", [128, 3 * 10 * 128], BF16)   # PE dw5 taps: pw (x) d, blockdiag
    din("alpha_ms", [128, 3], F32)           # Prelu alpha for ms mask
    din("idm", [128, 128], BF16)             # identity (acc folds)
    din("pjw", [128, 3 * 128], BF16)
    din("pjb", [128, 1], F32)
    din("m1w", [128, 6 * 128], BF16)
    din("m1b", [128, 6], F32)
    din("m1sq", [128, 6], F32)               # HSW_A*m1b + HSW_B
    din("dw3", [128, 54], F32)
    din("diag3", [128, 6 * 4 * 128], BF16)   # PE dw3 taps: diag weights
    din("m2b", [128, 6], F32)
    din("m2sq", [128, 6], F32)               # HSW_A*m2b + HSW_B
    din("m3w", [128, 6 * 128], BF16)
    din("m3b", [128, 1], F32)
    din("rr", [128, 66, 128], F32)     # ref rows lr=-1..64 fp32
    din("rb", [128, 66, 128], BF16)    # same in bf16
    d["out"] = nc.dram_tensor("out", [128, 64, 128], F32,
                              kind="ExternalOutput").ap()
    with tile.TileContext(nc) as tc:
        _emit(nc, tc, d)
    nc.compile()
    return nc


def _emit(nc, tc, d):
    import contextlib
    ctx = contextlib.ExitStack()
    with ctx:
        wp = ctx.enter_context(tc.tile_pool(name="wp", bufs=1))
        dram = ctx.enter_context(tc.tile_pool(name="dram", bufs=1, space="DRAM"))

        def wtile(name, shape, dt):
            t = wp.tile(shape, dt, tag=name, name=name)
            nc.sync.dma_start(out=t, in_=d[name])
            return t

        wc = wtile("wc", [128, 27 * 128], BF16)
        cb = wtile("cb", [128, 3], F32)
        cm = wtile("cm", [128, 3], F32)
        dw5 = wtile("dw5", [128, 75], F32)
        pww = wtile("pww", [128, 3 * 128], BF16)
        wblk5 = wtile("wblk5", [128, 3 * 10 * 128], BF16)
        alpha_ms = wtile("alpha_ms", [128, 3], F32)
        idm = wtile("idm", [128, 128], BF16)
        pjw = wtile("pjw", [128, 3 * 128], BF16)
        pjb = wtile("pjb", [128, 1], F32)
        m1w = wtile("m1w", [128, 6 * 128], BF16)
        m1b = wtile("m1b", [128, 6], F32)
        m1sq = wtile("m1sq", [128, 6], F32)
        dw3 = wtile("dw3", [128, 54], F32)
        diag3 = wtile("diag3", [128, 6 * 4 * 128], BF16)
        m2b = wtile("m2b", [128, 6], F32)
        m2sq = wtile("m2sq", [128, 6], F32)
        m3w = wtile("m3w", [128, 6 * 128], BF16)
        m3b = wtile("m3b", [128, 1], F32)
        ones = wp.tile([128, 1], BF16, tag="ones", name="ones")
        nc.vector.memset(ones, 1.0)
        epsb = wp.tile([128, 1], F32, tag="epsb", name="epsb")
        nc.vector.memset(epsb, ATT_EPS)

        att_dram = [dram.tile([96, 66 * 128], BF16, tag=f"attd{a}", name=f"attd{a}")
                    for a in range(3)]
        attf_dram = dram.tile([128, 66 * 128], BF16, tag="attfd", name="attfd")
        h1_dram = [dram.tile([128, 66, 130], BF16, tag=f"h1d{t}", name=f"h1d{t}")
                   for t in range(6)]
        rscr = dram.tile([36, 512], BF16, tag="rscr", name="rscr")
        cc_in = [dram.tile([128, 129], F32, tag=f"cc_in{h}", name=f"cc_in{h}")
                 for h in range(2)]
        cc_out = [dram.tile([128, 129], F32, tag=f"cc_out{h}", name=f"cc_out{h}")
                  for h in range(2)]

        attbp = ctx.enter_context(tc.tile_pool(name="attbp", bufs=1))
        ctx2 = contextlib.ExitStack()   # closes after P6 (frees qkv+ms)
        qkvp = ctx2.enter_context(tc.tile_pool(name="qkvp", bufs=3))
        msp = ctx2.enter_context(tc.tile_pool(name="msp", bufs=1))
        ms = msp.tile([128, 66, 384], BF16, tag="ms", name="ms")

        # ---- P1: qkv convs ----
        qkv = []
        with tc.tile_pool(name="xp", bufs=2) as xp, \
             tc.tile_pool(name="cps", bufs=2, space="PSUM") as cps:
            xr = xp.tile([128, 72, 130], BF16, tag="x", name="xr")
            xo = xp.tile([128, 72, 130], BF16, tag="x", name="xo")
            nc.sync.dma_start(out=xr, in_=d["xr"])
            nc.sync.dma_start(out=xo, in_=d["xo"])
            for j in range(3):
                qt = qkvp.tile([128, 70, 132], BF16, tag="qkv", name=f"qkv{j}")
                nc.vector.memset(qt, 0.0)
                src = xr if j == 0 else xo
                for (r0, nr) in _row_groups(70, 8):
                    ps = cps.tile([128, 8 * 128], F32, tag="cps", name="cps")
                    psv = ps[:, 0:nr * 128].rearrange("p (r w) -> p r w", w=128)
                    for (sr, sn) in _subs(nr):
                        first = True
                        for dy in range(3):
                            for dx in range(3):
                                k = (j * 9 + dy * 3 + dx) * 128
                                nc.tensor.matmul(
                                    psv[:, sr:sr + sn, :],
                                    wc[:, k:k + 128],
                                    src[:, r0 + sr + dy:r0 + sr + dy + sn,
                                        dx:dx + 128],
                                    start=first, stop=(dy == 2 and dx == 2))
                                first = False
                    nc.scalar.activation(
                        out=qt[:, r0:r0 + nr, 2:130], in_=psv[:, 0:nr, :],
                        func=AF.Identity, bias=cb[:, j:j + 1], scale=1.0)
                nc.vector.memset(qt[:, 0:3, :], 0.0)
                qkv.append(qt)

        # ---- pipelined region: dw5/ms bands + P4 transposes & kv ----
        with tc.tile_pool(name="kps", bufs=1, space="PSUM") as kps, \
             tc.tile_pool(name="pps", bufs=2, space="PSUM") as pps, \
             tc.tile_pool(name="accp", bufs=3) as accp, \
             tc.tile_pool(name="stgp", bufs=3) as stgp, \
             tc.tile_pool(name="kvcp", bufs=4) as kvcp, \
             tc.tile_pool(name="mtp", bufs=4) as mtp:
            kvT = [kps.tile([128, 132], F32, tag=f"kvT{h}", name=f"kvT{h}")
                   for h in range(2)]
            nkv = [0, 0]  # matmul counters per head (start/stop flags)

            def kv_group(h, mt, k):
                # mt: [128, 12, 128] transposed 4 rows (3 tile-blocks each)
                # gather k-cols / v-cols ((r, g) collapses: 16*24 = 384)
                kc = kvcp.tile([128, 4, 128], BF16, tag="kc", name="kc")
                vc = kvcp.tile([128, 4, 132], BF16, tag="vc", name="vc")
                kin = bass.AP(tensor=mt.tensor, offset=mt.offset + 8,
                              ap=[mt.ap[0], [384, 4], [24, 16], [1, 8]])
                vin = bass.AP(tensor=mt.tensor, offset=mt.offset + 16,
                              ap=[mt.ap[0], [384, 4], [24, 16], [1, 8]])
                nc.vector.tensor_copy(
                    out=kc.rearrange("p r (g e) -> p r g e", e=8), in_=kin)
                nc.vector.tensor_copy(
                    out=vc[:, :, 0:128].rearrange(
                        "p r (g e) -> p r g e", e=8), in_=vin)
                nc.vector.memset(vc[:, :, 128:129], 1.0)
                for rb in range(4):
                    first = (nkv[h] == 0)
                    last = (nkv[h] == 63)
                    nc.tensor.matmul(kvT[h][:, 0:129], kc[:, rb, :],
                                     vc[:, rb, 0:129], start=first, stop=last)
                    nkv[h] += 1

            # lhsT_att staging (memset early; scatter DMAs fill per head)
            lhsT_att = {}
            for a in ATT_SRCS:
                for S in ATT_SRCS[a]:
                    st = wp.tile([128, 108], F32, tag=f"lst{a}_{S}",
                                 name=f"lst{a}_{S}")
                    nc.vector.memset(st, 0.0)
                    bt = wp.tile([128, 108], BF16, tag=f"lat{a}_{S}",
                                 name=f"lat{a}_{S}")
                    lhsT_att[(a, S)] = (st, bt)
            compR = [None, None]

            def head_reduce(h):
                # kv partial -> pairwise AllReduce -> scatter into lhsT_att
                comp = wp.tile([128, 129], F32, tag=f"comp{h}", name=f"comp{h}")
                nc.vector.tensor_copy(out=comp, in_=kvT[h][:, 0:129])
                nc.sync.dma_start(out=cc_in[h][:], in_=comp)
                nc.gpsimd.collective_compute(
                    "AllReduce", Alu.add,
                    replica_groups=[[0, 1], [2, 3], [4, 5], [6, 7]],
                    ins=[cc_in[h].opt()], outs=[cc_out[h].opt()])
                cr = wp.tile([128, 129], F32, tag=f"compR{h}", name=f"compR{h}")
                nc.sync.dma_start(out=cr, in_=cc_out[h][:])
                compR[h] = cr
                engs = [nc.sync, nc.scalar, nc.gpsimd]
                for g in range(16 * h, 16 * h + 16):
                    a, gl9 = g // 12, g % 12
                    gl = g % 16
                    S, row0 = q_chan(g, 0) // 128, q_chan(g, 0) % 128
                    st = lhsT_att[(a, S)][0]
                    L = st.rearrange("p (dd gl) -> p dd gl", gl=12)
                    eng = engs[g % 3]
                    eng.dma_start(
                        out=L[row0:row0 + 8, 0:8, gl9:gl9 + 1],
                        in_=cr[8 * gl:8 * gl + 8, 8 * gl:8 * gl + 8])
                    eng.dma_start(
                        out=L[row0:row0 + 8, 8:9, gl9:gl9 + 1],
                        in_=cr[8 * gl:8 * gl + 8, 128:129])
                for (a, S), (st, bt) in lhsT_att.items():
                    if (S < 3) == (h == 0):
                        nc.vector.tensor_copy(out=bt, in_=st)
                        lhsT_att[(a, S)] = (st, bt)

            # P4 head0 (qkv channels), pipelined right after P1
            for k in range(16):
                stg = stgp.tile([128, 4, 384], BF16, tag="stg", name="stg")
                for t in range(3):
                    nc.vector.tensor_scalar_max(
                        out=stg[:, :, 128 * t:128 * t + 128],
                        in0=qkv[t][:, 3 + 4 * k:7 + 4 * k, 2:130],
                        scalar1=cm[:, t:t + 1])
                mt = mtp.tile([128, 12, 128], BF16, tag="mt", name="mtq")
                eng = nc.sync if k % 2 == 0 else nc.scalar
                eng.dma_start_transpose(mt, stg.rearrange("p r c -> p (r c)"))
                kv_group(0, mt, k)
            head_reduce(0)

            # dw5/ms bands + P4 head1
            ms_done_groups = 0

            def emit_ms_p4(upto_row):
                nonlocal ms_done_groups
                while ms_done_groups < 16 and 5 + 4 * ms_done_groups <= upto_row:
                    k = ms_done_groups
                    mt = mtp.tile([128, 12, 128], BF16, tag="mt", name="mtm")
                    eng = nc.sync if k % 2 == 0 else nc.scalar
                    eng.dma_start_transpose(
                        mt, ms[:, 1 + 4 * k:5 + 4 * k, :].rearrange(
                            "p r c -> p (r c)"))
                    kv_group(1, mt, k)
                    ms_done_groups += 1

            for (r0, nr) in _row_groups(66, 8):
                for t in range(3):
                    # DVE acc chain (17 taps, fused mul-add)
                    acc = accp.tile([128, 8, 128], BF16, tag="acc", name="acc")
                    first = True
                    for (dy, dx) in DW5_DVE:
                        w_ap = dw5[:, t * 25 + 5 * dy + dx:t * 25 + 5 * dy + dx + 1]
                        win = qkv[t][:, r0 + dy:r0 + dy + nr, dx:dx + 128]
                        if first:
                            nc.vector.tensor_scalar_mul(
                                out=acc[:, 0:nr, :], in0=win, scalar1=w_ap)
                            first = False
                        else:
                            nc.vector.scalar_tensor_tensor(
                                out=acc[:, 0:nr, :], in0=win, scalar=w_ap,
                                in1=acc[:, 0:nr, :], op0=Alu.mult, op1=Alu.add)
                    ps = pps.tile([128, 8 * 128], F32, tag="pps", name="pps")
                    psv = ps[:, 0:nr * 128].rearrange("p (r w) -> p r w", w=128)
                    for (sr, sn) in _subs(nr):
                        first = True
                        for i, (dy, dx) in enumerate(DW5_PE):
                            nc.tensor.matmul(
                                psv[:, sr:sr + sn, :],
                                wblk5[:, (t * 10 + i) * 128:(t * 10 + i) * 128 + 128],
                                qkv[t][:, r0 + sr + dy:r0 + sr + dy + sn,
                                       dx:dx + 128],
                                start=first, stop=False)
                            first = False
                        nc.tensor.matmul(psv[:, sr:sr + sn, :],
                                         pww[:, t * 128:t * 128 + 128],
                                         acc[:, sr:sr + sn, :],
                                         start=False, stop=True)
                    # Prelu epilogue: relu for q/k positions, identity for v
                    nc.scalar.activation(
                        out=ms[:, r0:r0 + nr, 128 * t:128 * t + 128],
                        in_=psv[:, 0:nr, :], func=AF.Prelu,
                        alpha=alpha_ms[:, t:t + 1])
                emit_ms_p4(r0 + nr)
            emit_ms_p4(66)
            head_reduce(1)
            lhsT_att = {k: bt for k, (st, bt) in lhsT_att.items()}

        # mask qkv in place for P6 q-side rhs (after dw5 consumed raw qkv)
        for t in range(3):
            nc.vector.tensor_scalar_max(out=qkv[t], in0=qkv[t],
                                        scalar1=cm[:, t:t + 1])

        # ---- P6: att9 matmuls + division -> att_dram ----
        with tc.tile_pool(name="aps", bufs=4, space="PSUM") as aps, \
             tc.tile_pool(name="dnp", bufs=2) as dnp:
            for (c0, cn) in _subs(66):
                cw = cn * 128

                def att_rhs(S):
                    if S < 3:
                        return qkv[S][:, 2 + c0:2 + c0 + cn, 2:130]
                    return ms[:, c0:c0 + cn,
                              128 * (S - 3):128 * (S - 3) + 128]

                psl = []
                for a in range(3):
                    ps = aps.tile([108, 512], F32, tag="aps", name="aps")
                    srcs = ATT_SRCS[a]
                    for i, S in enumerate(srcs):
                        nc.tensor.matmul(ps[:, 0:cw], lhsT_att[(a, S)],
                                         att_rhs(S), start=(i == 0),
                                         stop=(i == len(srcs) - 1))
                    psl.append(ps)
                den = dnp.tile([128, 512], F32, tag="den", name="den")
                nc.vector.memset(den, 1.0)
                for a in range(3):
                    nc.scalar.activation(out=den[32 * a:32 * a + 12, 0:cw],
                                         in_=psl[a][96:108, 0:cw],
                                         func=AF.Identity,
                                         bias=epsb[0:12, 0:1], scale=1.0)
                rec = dnp.tile([128, 512], BF16, tag="rec", name="rec")
                with nc.allow_low_precision(reason="den recip to bf16"):
                    nc.vector.reciprocal(out=rec[0:76, 0:cw],
                                         in_=den[0:76, 0:cw])
                for a in range(3):
                    nc.gpsimd.dma_start(out=rscr[12 * a:12 * a + 12, 0:cw],
                                        in_=rec[32 * a:32 * a + 12, 0:cw])
                for a in range(3):
                    dexp = dnp.tile([96, 512], BF16, tag="dexp", name="dexp")
                    src = bass.AP(tensor=rscr.tensor,
                                  offset=rscr.offset + 12 * a * 512,
                                  ap=[[0, 8], [512, 12], [1, cw]])
                    nc.gpsimd.dma_start(out=dexp[:, 0:cw], in_=src)
                    attc = dnp.tile([96, 512], BF16, tag="attc", name="attc")
                    nc.vector.tensor_mul(out=attc[:, 0:cw], in0=dexp[:, 0:cw],
                                         in1=psl[a][0:96, 0:cw])
                    nc.sync.dma_start(
                        out=att_dram[a][:, c0 * 128:c0 * 128 + cw],
                        in_=attc[:, 0:cw])

        ctx2.close()

        # ---- P7: attn proj + residual ----
        attB = attbp.tile([128, 66, 128], BF16, tag="attB", name="attB")
        with tc.tile_pool(name="jps", bufs=2, space="PSUM") as jps, \
             tc.tile_pool(name="arp", bufs=4) as arp, \
             tc.tile_pool(name="afp", bufs=2) as afp:
            for (r0, nr) in _row_groups(66):
                ps = jps.tile([128, 16 * 128], F32, tag="jps", name="jps")
                for (sr, sn) in _subs(nr):
                    cw = sn * 128
                    col0 = (r0 + sr) * 128
                    for a in range(3):
                        at = arp.tile([96, 512], BF16, tag="arp", name="arp")
                        nc.sync.dma_start(out=at[:, 0:cw],
                                          in_=att_dram[a][:, col0:col0 + cw])
                        nc.tensor.matmul(
                            ps[:, sr * 128:sr * 128 + cw],
                            pjw[0:96, a * 128:a * 128 + 128], at[:, 0:cw],
                            start=(a == 0), stop=(a == 2))
                nc.scalar.activation(
                    out=attB[:, r0:r0 + nr, :], in_=ps[:, 0:nr * 128],
                    func=AF.Identity, bias=pjb[:, 0:1], scale=1.0)
                af = afp.tile([128, 16 * 128], BF16, tag="afp", name="afp")
                rb = afp.tile([128, 16, 128], BF16, tag="rbl", name="rbl")
                nc.sync.dma_start(out=rb[:, 0:nr, :],
                                  in_=d["rb"][:, r0:r0 + nr, :])
                nc.vector.tensor_add(
                    out=af[:, 0:nr * 128],
                    in0=attB[:, r0:r0 + nr, :].rearrange("p r w -> p (r w)"),
                    in1=rb[:, 0:nr, :].rearrange("p r w -> p (r w)"))
                nc.sync.dma_start(
                    out=attf_dram[:, r0 * 128:(r0 + nr) * 128],
                    in_=af[:, 0:nr * 128])

        # ---- P8: mb1 + hswish -> h1_dram ----
        with tc.tile_pool(name="m1ps", bufs=2, space="PSUM") as m1ps, \
             tc.tile_pool(name="hwp", bufs=2) as hwp:
            for (r0, nr) in _row_groups(66):
                afl = hwp.tile([128, 16 * 128], BF16, tag="afl", name="afl")
                nc.sync.dma_start(out=afl[:, 0:nr * 128],
                                  in_=attf_dram[:, r0 * 128:(r0 + nr) * 128])
                aflv = afl[:, 0:nr * 128].rearrange("p (r w) -> p r w", w=128)
                for t in range(6):
                    ps = m1ps.tile([128, 16 * 128], F32, tag="m1ps", name="m1ps")
                    for (sr, sn) in _subs(nr):
                        nc.tensor.matmul(
                            ps[:, sr * 128:(sr + sn) * 128],
                            m1w[:, t * 128:t * 128 + 128],
                            aflv[:, sr:sr + sn, :], start=True, stop=True)
                    pw_ = ps[:, 0:nr * 128]
                    rx = hwp.tile([128, 16 * 128], BF16, tag="rx", name="rx")
                    sq = hwp.tile([128, 16 * 128], BF16, tag="sq", name="sq")
                    nc.scalar.activation(out=rx[:, 0:nr * 128], in_=pw_,
                                         func=AF.Relu, bias=m1b[:, t:t + 1],
                                         scale=1.0)
                    nc.scalar.activation(out=sq[:, 0:nr * 128], in_=pw_,
                                         func=AF.Square,
                                         bias=m1sq[:, t:t + 1], scale=HSW_A)
                    hw = hwp.tile([128, 16, 130], BF16, tag="hw", name="hw")
                    nc.vector.memset(hw[:, 0:nr, 0:1], 0.0)
                    nc.vector.memset(hw[:, 0:nr, 129:130], 0.0)
                    nc.vector.scalar_tensor_tensor(
                        out=hw[:, 0:nr, 1:129],
                        in0=sq[:, 0:nr * 128].rearrange(
                            "p (r w) -> p r w", w=128),
                        scalar=HSW_C,
                        in1=rx[:, 0:nr * 128].rearrange(
                            "p (r w) -> p r w", w=128),
                        op0=Alu.add, op1=Alu.min)
                    if r0 == 0:
                        nc.vector.memset(hw[:, 0:1, :], 0.0)  # lr=-1 edge
                    nc.sync.dma_start(out=h1_dram[t][:, r0:r0 + nr, :],
                                      in_=hw[:, 0:nr, :])

        # ---- P9: dw3 + hswish + mb3 + final adds ----
        with tc.tile_pool(name="m3ps", bufs=1, space="PSUM") as m3ps, \
             tc.tile_pool(name="d3ps", bufs=2, space="PSUM") as d3ps, \
             tc.tile_pool(name="h2p", bufs=2) as h2p, \
             tc.tile_pool(name="h2w", bufs=2) as h2w, \
             tc.tile_pool(name="osp", bufs=2) as osp:
            for q in range(4):
                ps = m3ps.tile([128, 16 * 128], F32, tag="m3ps", name="m3ps")
                for t in range(6):
                    hv = h2w.tile([128, 18, 130], BF16, tag="hv", name="hv")
                    nc.sync.dma_start(out=hv,
                                      in_=h1_dram[t][:, 16 * q:16 * q + 18, :])
                    acc = h2p.tile([128, 16, 128], BF16, tag="acc3", name="acc3")
                    first = True
                    for (dy, dx) in DW3_DVE:
                        w_ap = dw3[:, t * 9 + 3 * dy + dx:t * 9 + 3 * dy + dx + 1]
                        win = hv[:, dy:dy + 16, dx:dx + 128]
                        if first:
                            nc.vector.tensor_scalar_mul(out=acc, in0=win,
                                                        scalar1=w_ap)
                            first = False
                        else:
                            nc.vector.scalar_tensor_tensor(
                                out=acc, in0=win, scalar=w_ap, in1=acc,
                                op0=Alu.mult, op1=Alu.add)
                    for (sr, sn) in _subs(16):
                        ps3 = d3ps.tile([128, 4 * 128], F32, tag="d3ps",
                                        name="d3ps")
                        first = True
                        for i, (dy, dx) in enumerate(DW3_PE):
                            nc.tensor.matmul(
                                ps3[:, 0:sn * 128].rearrange(
                                    "p (r w) -> p r w", w=128),
                                diag3[:, (t * 4 + i) * 128:(t * 4 + i) * 128 + 128],
                                hv[:, sr + dy:sr + dy + sn, dx:dx + 128],
                                start=first, stop=False)
                            first = False
                        nc.tensor.matmul(
                            ps3[:, 0:sn * 128].rearrange(
                                "p (r w) -> p r w", w=128),
                            idm, acc[:, sr:sr + sn, :],
                            start=False, stop=True)
                        rx = h2p.tile([128, 4 * 128], BF16, tag="rx3", name="rx3")
                        sq = h2p.tile([128, 4 * 128], BF16, tag="sq3", name="sq3")
                        nc.scalar.activation(out=rx[:, 0:sn * 128],
                                             in_=ps3[:, 0:sn * 128],
                                             func=AF.Relu, bias=m2b[:, t:t + 1],
                                             scale=1.0)
                        nc.scalar.activation(out=sq[:, 0:sn * 128],
                                             in_=ps3[:, 0:sn * 128],
                                             func=AF.Square,
                                             bias=m2sq[:, t:t + 1], scale=HSW_A)
                        h2f = h2p.tile([128, 4 * 128], BF16, tag="h2f", name="h2f")
                        nc.vector.scalar_tensor_tensor(
                            out=h2f[:, 0:sn * 128], in0=sq[:, 0:sn * 128],
                            scalar=HSW_C, in1=rx[:, 0:sn * 128],
                            op0=Alu.add, op1=Alu.min)
                        nc.tensor.matmul(
                            ps[:, sr * 128:(sr + sn) * 128],
                            m3w[:, t * 128:t * 128 + 128],
                            h2f[:, 0:sn * 128],
                            start=(t == 0), stop=(t == 5))
                o1 = osp.tile([128, 16 * 128], F32, tag="o1", name="o1")
                nc.scalar.activation(out=o1, in_=ps, func=AF.Identity,
                                     bias=m3b[:, 0:1], scale=1.0)
                rq = osp.tile([128, 16, 128], F32, tag="rq", name="rq")
                nc.sync.dma_start(out=rq,
                                  in_=d["rr"][:, 16 * q + 1:16 * q + 17, :])
                nc.vector.tensor_add(out=o1, in0=o1,
                                     in1=rq.rearrange("p r w -> p (r w)"))
                nc.vector.tensor_add(
                    out=o1, in0=o1,
                    in1=attB[:, 16 * q + 1:16 * q + 17, :].rearrange(
                        "p r w -> p (r w)"))
                nc.sync.dma_start(out=d["out"][:, 16 * q:16 * q + 16, :],
                                  in_=o1.rearrange("p (r w) -> p r w", w=128))


# ====================== host side ======================

def _prep_shared(inp):
    f32 = np.float32
    out = {}
    for s in (0, 1):
        w = {}
        wc = np.zeros((128, 27 * 128), f32)
        for j, cw in enumerate((inp["wq"], inp["wk"], inp["wv"])):
            for dy in range(3):
                dyy = 2 - dy if s == 1 else dy
                for dx in range(3):
                    k = (j * 9 + dy * 3 + dx) * 128
                    wc[:, k:k + 128] = cw[:, :, dyy, dx].T
        w["wc"] = wc.astype(BF)
        w["cb"] = np.stack([inp["bq"], inp["bk"], inp["bv"]], 1).astype(f32)
        m = np.arange(384)
        w["cm"] = np.where((m % 24) < 16, 0.0, -1e9).astype(f32).reshape(3, 128).T.copy()
        w["alpha_ms"] = np.where((m % 24) < 16, 0.0, 1.0).astype(f32).reshape(3, 128).T.copy()
        dw5 = np.zeros((128, 75), f32)
        for t in range(3):
            for tap in range(25):
                dy, dx = tap // 5, tap % 5
                dyy = 4 - dy if s == 1 else dy
                dw5[:, t * 25 + tap] = inp["agg_dw_w"][128 * t:128 * t + 128, 0, dyy, dx]
        w["dw5"] = dw5
        pw = inp["agg_pw_w"][:, :, 0, 0]
        pww = np.zeros((128, 3 * 128), f32)
        for mc in range(384):
            t, o = mc // 128, mc % 128
            g8 = (o // 8) * 8
            pww[g8:g8 + 8, t * 128 + o] = pw[mc]
        w["pww"] = pww.astype(BF)
        # PE dw5 taps: blockdiag weights pw[o,i]*d[i,tap]
        wblk5 = np.zeros((128, 3 * 10 * 128), f32)
        for t in range(3):
            for i_t, (dy, dx) in enumerate(DW5_PE):
                tap = 5 * dy + dx
                col0 = (t * 10 + i_t) * 128
                # lhsT[i, o] = pw[o_global, i] * d[i_global, tap]
                wblk5[:, col0:col0 + 128] = (
                    pww[:, t * 128:t * 128 + 128]
                    * dw5[:, t * 25 + tap:t * 25 + tap + 1])
        w["wblk5"] = wblk5.astype(BF)
        w["idm"] = np.eye(128, dtype=f32).astype(BF)
        s1 = inp["bn1_g"] / np.sqrt(inp["bn1_v"] + BN_EPS)
        b1 = inp["bn1_b"] - inp["bn1_m"] * s1
        Wp = inp["attn_proj_w"][:, :, 0, 0] * s1[:, None]
        pjw = np.zeros((128, 3 * 128), f32)
        for g in range(32):
            a, gl9 = g // 12, g % 12
            for dd in range(8):
                pjw[12 * dd + gl9, a * 128:a * 128 + 128] = Wp[:, 8 * g + dd]
        w["pjw"] = pjw.astype(BF)
        w["pjb"] = b1.reshape(128, 1).astype(f32)
        m1w = np.zeros((128, 6 * 128), f32)
        for t in range(6):
            m1w[:, t * 128:t * 128 + 128] = inp["mb1_w"][128 * t:128 * t + 128, :, 0, 0].T
        w["m1w"] = m1w.astype(BF)
        m1b = inp["mb1_b"].reshape(6, 128).T.copy().astype(f32)
        w["m1b"] = m1b
        w["m1sq"] = (HSW_A * m1b + HSW_B).astype(f32)
        dw3 = np.zeros((128, 54), f32)
        for t in range(6):
            for tap in range(9):
                dy, dx = tap // 3, tap % 3
                dyy = 2 - dy if s == 1 else dy
                dw3[:, t * 9 + tap] = inp["mb2_w"][128 * t:128 * t + 128, 0, dyy, dx]
        w["dw3"] = dw3
        diag3 = np.zeros((128, 6 * 4 * 128), f32)
        for t in range(6):
            for i_t, (dy, dx) in enumerate(DW3_PE):
                tap = 3 * dy + dx
                col0 = (t * 4 + i_t) * 128
                diag3[:, col0:col0 + 128] = np.diag(dw3[:, t * 9 + tap])
        w["diag3"] = diag3.astype(BF)
        m2b = inp["mb2_b"].reshape(6, 128).T.copy().astype(f32)
        w["m2b"] = m2b
        w["m2sq"] = (HSW_A * m2b + HSW_B).astype(f32)
        s2 = inp["bn2_g"] / np.sqrt(inp["bn2_v"] + BN_EPS)
        b2 = inp["bn2_b"] - inp["bn2_m"] * s2
        W3 = inp["mb3_w"][:, :, 0, 0] * s2[:, None]
        m3w = np.zeros((128, 6 * 128), f32)
        for t in range(6):
            m3w[:, t * 128:t * 128 + 128] = W3[:, 128 * t:128 * t + 128].T
        w["m3w"] = m3w.astype(BF)
        w["m3b"] = b2.reshape(128, 1).astype(f32)
        out[s] = w
    return out


def _prep_core(inp, b, s):
    f32 = np.float32
    ref = inp["ref_features"][b]
    oth = inp["other_features"][b]
    if s == 1:
        ref = ref[:, ::-1, :]
        oth = oth[:, ::-1, :]
    xr = np.zeros((128, 72, 130), f32)
    xo = np.zeros((128, 72, 130), f32)
    xr[:, 4:72, 1:129] = ref[:, 0:68, :]
    xo[:, 4:72, 1:129] = oth[:, 0:68, :]
    rr = np.zeros((128, 66, 128), f32)
    rr[:, 1:66, :] = ref[:, 0:65, :]
    return {"xr": xr.astype(BF), "xo": xo.astype(BF),
            "rr": rr, "rb": rr.astype(BF)}


def kernel(**inputs):
    inp = {k: np.asarray(v) for k, v in inputs.items()}
    if "nc" not in _CACHE:
        _CACHE["nc"] = build_program()
    nc = _CACHE["nc"]
    ws = _prep_shared(inp)
    in_maps = []
    for c in range(NCORES):
        b, s = c // 2, c % 2
        m = dict(ws[s])
        m.update(_prep_core(inp, b, s))
        in_maps.append(m)
    res = bass_utils.run_bass_kernel_spmd(nc, in_maps,
                                          core_ids=list(range(NCORES)))
    out = np.zeros((4, 128, 128, 128), np.float32)
    for c in range(NCORES):
        b, s = c // 2, c % 2
        o = res.results[c]["out"]
        if s == 1:
            o = o[:, ::-1, :]
        out[b, :, 64 * s:64 * s + 64, :] = o
    return out
